# revision 62
# baseline (speedup 1.0000x reference)
"""Trainium2 Bass kernel for the CO2-electrolysis surrogate model.

Contract: kernel(**inputs) takes FULL unsharded inputs (x [16384,5], MLP
weights, kinetic params i0/alpha) and returns the FULL [16384,2] output.
Internally: batch is sharded 2048-per-core across 8 NeuronCores (pure data
parallel).

Design (v4):
- x is transposed on the host into xT [5, 2048] (column c = t*128+p holds
  sample s = p*16+t) so the MLP needs no PE transposes. zlt (= x[:,3]), the
  search/kinetics constant table (functions of i0/alpha), and b4 ride in one
  packed [128, 91] input; 4 input DMAs per rep.
- Matmul inputs are float32r end-to-end (1 cycle/row at N=512 vs 4 for fp32).
- Relu reads two-bank PSUM tiles [128, 2x512], split DVE/ACT (tensor_scalar
  is not implemented on the Pool engine).
- Reps are emitted in groups of G: the MLPs run per rep (interleaved
  emission), then ONE grouped tail runs the per-sample physics for all G
  reps with a leading rep axis in every tile - 3x fewer instructions and
  semaphore hops on the serial search chain.
- The voltage search keeps multiplicative state AE[p,r,t,k] = 1/i_kin at the
  current virtual-grid index b; each 4-ary step probes b+{1,2,3}*s via baked
  factor multiplies and updates AE *= exp(sc*s*u) with u = #successful
  probes (no exp of per-sample args; b stays off the critical path).
- The parameter section uses only {exp, ln, abs, relu, copy}, all inside
  activation-table set 6 (natural_log_exp_and_others), loaded once.
  Identities: 1/(1-sigmoid(l)) = 1+e^l, sigmoid(l)^-1.5 = exp(1.5*ln(1+e^-l)).
- b <= 998 always (i_tot(999) < 1e-3 << target for any theta <= 1), so the
  refine step only needs the b < 0 boundary case.
"""

import sys

for _p in ("/opt/trn_rl_repo", "/opt/pypackages"):
    if _p not in sys.path:
        sys.path.insert(0, _p)

import math

import numpy as np

import concourse.bacc as bacc
import concourse.bass as bass
import concourse.tile as tile
from concourse import mybir

F32 = mybir.dt.float32
F32R = mybir.dt.float32r
I32 = mybir.dt.int32
AF = mybir.ActivationFunctionType
OP = mybir.AluOpType

# ---- problem constants (match reference.py) ----
N = 16384
NCORES = 8
NPC = N // NCORES            # 2048 samples per core
NT = NPC // 128              # 16 tiles of 128 samples
HID = 64
GRID = 1000
VMIN, VMAX = -1.25, 0.0
I_TARGET = 200.0
F_CONST = 96485.33
RT = 8.314 * 298.15
D_CO2 = 1.91e-9
C_CO2 = 34.0
E_EQ = (-0.11, 0.08, 0.0)
N_ELEC_CO2 = (2.0, 12.0)
DV = (VMAX - VMIN) / (GRID - 1)
FRT = F_CONST / RT
STEPS = [256, 64, 16, 4, 1]   # 4-ary climb over virtual 1024-grid, b in [-1,1022]
GRPSZ = 3                     # reps per grouped tail
_DBG_STAGE = 0

# blobz layout: [zlt(16) | blob(NBLOB) | b4(6)]
BL0 = NT                      # blob base column inside blobz
LNF_C = 45
CAE_C = 60
F1_C = 63
LN4_C = 66
CIL_C = 67
NBLOB = 70


def _make_blob_row(i0, alpha):
    """Search/kinetics constants [NBLOB] f32 (functions of i0/alpha)."""
    i0 = np.asarray(i0, np.float64)
    alpha = np.asarray(alpha, np.float64)
    sc = [float(alpha[k] * FRT * DV) for k in range(3)]
    t0 = [float(alpha[k] * FRT * (VMIN - E_EQ[k])) for k in range(3)]
    cols = []
    for s in STEPS:                 # 0:45  probe factors exp(sc*j*s)
        for j in (1, 2, 3):
            for k in range(3):
                cols.append(np.exp(sc[k] * j * s))
    for s in STEPS:                 # 45:60 ln of climb factor: sc_k * s
        for k in range(3):
            cols.append(sc[k] * s)
    for k in range(3):              # 60:63 cAE: exp(t0-sc)/i0 (AE at b=-1, pre 1/theta)
        cols.append(np.exp(t0[k] - sc[k]) / float(i0[k]))
    for k in range(3):              # 63:66 f1: one-step factor exp(sc)
        cols.append(np.exp(sc[k]))
    cols.append(math.log(4e-8))     # 66    bias for the r/Kdl exp
    # 67:70  1/i_lim prefactors (3rd species: H2, not transport-limited -> 0)
    for nk in N_ELEC_CO2:
        cols.append(1.0 / (float(np.float32(np.float32(nk) * F_CONST))
                           * C_CO2 * D_CO2))
    cols.append(0.0)
    row = np.asarray(cols, np.float32)
    assert row.size == NBLOB
    return row


class _Pools:
    pass


def _mk_pools(ctx, tc):
    p = _Pools()
    p.io = ctx.enter_context(tc.tile_pool(name="io", bufs=3))
    p.work = ctx.enter_context(tc.tile_pool(name="work", bufs=3))
    p.psum = ctx.enter_context(tc.tile_pool(name="psum", bufs=3, space="PSUM"))
    return p


def _mlp(tc, po, io, r, g, lat3, azlt3, shared, first):
    """Generator: per-rep DMAs + MLP; writes lat into lat3[:, r] and
    |zlt| into azlt3[:, r].  shared[0] collects the rep's blobsb tile."""
    nc = tc.nc
    xT_d, W1_d, pack_d, blobz_d, out_d = io

    if first:
        # lock the activation table to set 6 (natural_log_exp_and_others):
        # covers exp/ln/abs/relu/copy -> zero reloads for the whole program
        inst = mybir.InstLoadActFuncSet(
            name=nc.get_next_instruction_name(), act_func_set_id=6, ins=[], outs=[])
        nc.scalar.add_instruction(inst)

    xTsb = po.io.tile([5, 4, 512], F32R, name="xTsb")
    nc.sync.dma_start(xTsb, xT_d.rearrange("k (i n) -> k i n", i=4))
    packsb = po.io.tile([64, 137], F32R, name="packsb")
    nc.sync.dma_start(packsb, pack_d)
    W1sb = po.io.tile([5, 64], F32R, name="W1sb")
    nc.sync.dma_start(W1sb, W1_d)
    blobsb = po.io.tile([128, NT + NBLOB + 6], F32, name="blobsb")
    nc.sync.dma_start(blobsb, blobz_d)
    shared[r] = blobsb

    W2sb = packsb[:, 0:64]
    W3sb = packsb[:, 64:128]
    W4sb = packsb[:, 128:134]
    biases = [packsb[:, 134 + i:135 + i].bitcast(F32) for i in range(3)]

    def layer(W, movsrc, bias, name, eng):
        ps = [po.psum.tile([128, 2, 512], F32, tag="ps2", name=f"{name}ps{i}")
              for i in range(2)]
        for i in range(4):
            nc.tensor.matmul(ps[i // 2][0:64, i % 2, :], W, movsrc(i))
        h = po.work.tile([64, 4, 512], F32R, tag=name, name=name, bufs=3)
        for i in range(2):
            dst = h[:, 2 * i:2 * i + 2, :]
            src = ps[i][0:64, :, :]
            if eng[i] == "v":
                nc.vector.tensor_scalar(dst, src, bias, 0.0, OP.add, OP.max)
            else:
                nc.scalar.activation(dst, src, AF.Relu, bias=bias, scale=1.0)
        return h

    yield
    h1 = layer(W1sb, lambda i: xTsb[:, i, :], biases[0], "h1", "av")
    nc.scalar.activation(azlt3[:, r], blobsb[:, 0:NT], AF.Abs, scale=1.0)
    yield
    h2 = layer(W2sb, lambda i: h1[:, i, :], biases[1], "h2", "va")
    yield
    h3 = layer(W3sb, lambda i: h2[:, i, :], biases[2], "h3", "aa")
    yield
    h3f = h3.rearrange("p a c -> p (a c)")

    latps = po.psum.tile([128, 96], F32, tag="lat", name="latps", bufs=2)
    for t in range(NT):
        nc.tensor.matmul(latps[:, t * 6:(t + 1) * 6],
                         h3f[:, t * 128:(t + 1) * 128], W4sb)
    b4b = bass.AP(tensor=blobsb.tensor, offset=blobsb.offset + BL0 + NBLOB,
                  ap=[list(blobsb.ap[0]), [0, NT], [1, 6]])
    nc.vector.tensor_tensor(lat3[:, r], latps.rearrange("p (t j) -> p t j", j=6),
                            b4b, OP.add)


def _tail(tc, po, out_d, g, lat3, azlt3, blobsb):
    """Grouped per-sample physics for g reps: parameters, 4-ary climb,
    refine, FE output.  All tiles carry a leading rep axis of size g."""
    nc = tc.nc
    GNT = g * NT

    def bcol(c, n=3):
        """blob columns c..c+n as [p, r(bcast), t(bcast), k] AP"""
        return bass.AP(tensor=blobsb.tensor, offset=blobsb.offset + BL0 + c,
                       ap=[list(blobsb.ap[0]), [0, g * NT], [1, n]])

    if _DBG_STAGE == 1:
        nc.sync.dma_start(out_d.rearrange("(p t) c -> p t c", t=NT),
                          lat3[:, 0, :, 0:2])
        return

    def w3(name, dt=F32):
        return po.work.tile([128, g, NT], dt, tag=name, name=name)

    def w33(name):
        return po.work.tile([128, g, NT, 3], F32, tag=name, name=name)

    a1, e1i, a2, a3, d1, a4, Lt, s5, t6, mm, st, b = (
        w3(n) for n in ("a1", "e1i", "a2", "a3", "d1", "a4", "Lt",
                        "s5", "t6", "mm", "st", "b"))
    C_all, dk, T, iT, t7, AE = (
        w33(n) for n in ("C_all", "dk", "T", "iT", "t7", "AE"))
    Cfull = po.work.tile([128, 3, g, NT, 3], F32, tag="Cfull", name="Cfull")

    def sl(t):
        return t

    l3 = lat3
    nc.scalar.activation(sl(a1), l3[:, :, :, 1], AF.Exp, scale=-1.0)   # e^-l1
    nc.vector.reciprocal(sl(e1i), sl(a1))                              # e^l1
    nc.scalar.activation(sl(a2), sl(a1), AF.Ln, bias=1.0, scale=1.0)   # ln(1+e^-l1)
    nc.scalar.activation(sl(a3), sl(a2), AF.Exp, scale=1.5)            # eps^-1.5
    nc.gpsimd.tensor_tensor(sl(d1), l3[:, :, :, 0], l3[:, :, :, 2], OP.subtract)
    nc.scalar.activation(sl(a4), sl(d1), AF.Exp,
                         bias=blobsb[:, BL0 + LN4_C:BL0 + LN4_C + 1],
                         scale=1.0)                                    # r/Kdl
    nc.vector.scalar_tensor_tensor(sl(Lt), sl(e1i), 1.0, sl(azlt3),
                                   OP.add, OP.mult)
    nc.gpsimd.tensor_tensor(sl(s5), sl(a4), sl(Lt), OP.add)
    nc.gpsimd.tensor_tensor(sl(t6), sl(s5), sl(a3), OP.mult)
    # C_all[:, r, :, k] = t6 * cilim_k  (k=2 blob column is 0 -> H2 term)
    t6b = bass.AP(tensor=t6.tensor, offset=t6.offset,
                  ap=[list(t6.ap[0]), [1, g * NT], [0, 3]])
    nc.vector.tensor_tensor(sl(C_all), t6b, bcol(CIL_C), OP.mult)

    # softmax thetas -> AE init at b=-1
    nc.vector.reduce_max(sl(mm), l3[:, :, :, 3:6], axis=mybir.AxisListType.X,
                         opt_input=False)
    mmb = bass.AP(tensor=mm.tensor, offset=mm.offset,
                  ap=[list(mm.ap[0]), [1, g * NT], [0, 3]])
    nc.vector.tensor_tensor(sl(dk), l3[:, :, :, 3:6], mmb, OP.subtract)
    nc.scalar.activation(sl(T), sl(dk), AF.Exp, scale=2.0)
    nc.vector.reduce_sum(sl(st), sl(T), axis=mybir.AxisListType.X,
                         opt_input=False)
    nc.vector.reciprocal(sl(iT), sl(T))
    yield
    nc.vector.tensor_tensor(sl(t7), sl(iT), bcol(CAE_C), OP.mult)
    stb = bass.AP(tensor=st.tensor, offset=st.offset,
                  ap=[list(st.ap[0]), [1, g * NT], [0, 3]])
    nc.vector.tensor_tensor(sl(AE), sl(t7), stb, OP.mult)     # 1/i_kin at b=-1
    nc.vector.memset(sl(b), -1.0)
    # materialize C over the probe axis
    C_b = bass.AP(tensor=C_all.tensor, offset=C_all.offset,
                  ap=[list(C_all.ap[0]), [0, 3], [1, g * NT * 3]])
    nc.vector.tensor_copy(Cfull, C_b)
    yield

    if _DBG_STAGE == 2:
        o = po.work.tile([128, NT, 2], F32, name="dbg2")
        nc.vector.tensor_copy(o[:, :, 0], C_all[:, 0, :, 0])
        nc.vector.tensor_copy(o[:, :, 1], AE[:, 0, :, 0])
        nc.sync.dma_start(out_d.rearrange("(p t) c -> p t c", t=NT), o)
        return

    # ---------- 4-ary climb ----------
    AEb = bass.AP(tensor=AE.tensor, offset=AE.offset,
                  ap=[list(AE.ap[0]), [0, 3], [1, g * NT * 3]])
    for jj, s in enumerate(STEPS):
        s = float(s)
        AEp = po.work.tile([128, 3, g, NT, 3], F32, tag="AEp",
                           name=f"AEp{jj}", bufs=7)
        fstep = bass.AP(tensor=blobsb.tensor, offset=blobsb.offset + BL0 + 9 * jj,
                        ap=[list(blobsb.ap[0]), [3, 3], [0, g * NT], [1, 3]])
        nc.gpsimd.tensor_tensor(AEp, AEb, fstep, OP.mult)
        P = po.work.tile([128, 3, g, NT, 3], F32, tag="P", name=f"P{jj}",
                         bufs=7)
        nc.gpsimd.tensor_tensor(P, AEp, Cfull, OP.add)
        S = po.work.tile([128, 3, g, NT, 3], F32, tag="S", name=f"S{jj}",
                         bufs=7)
        nc.vector.reciprocal(S, P)
        itot = po.work.tile([128, 3, g, NT], F32, tag="it", name=f"it{jj}",
                            bufs=7)
        nc.vector.reduce_sum(itot, S, axis=mybir.AxisListType.X,
                             opt_input=False)
        yield
        # cp[p, r, t, j] = (i_tot at probe j >= target), transposed write
        cp = po.work.tile([128, g, NT, 3], F32, tag="cp", name=f"cp{jj}",
                          bufs=7)
        cpw = bass.AP(tensor=cp.tensor, offset=cp.offset,
                      ap=[list(cp.ap[0]), [1, 3], [3, g * NT]])
        nc.vector.tensor_scalar(cpw, itot, I_TARGET, None, OP.is_ge)
        # u = #successful probes; b += s*u; AE *= exp(sc*s*u)
        u = po.work.tile([128, g, NT], F32, tag="u", name=f"u{jj}", bufs=7)
        nc.vector.reduce_sum(sl(u), sl(cp), axis=mybir.AxisListType.X,
                             opt_input=False)
        nc.vector.scalar_tensor_tensor(sl(b), sl(u), s, sl(b), OP.mult, OP.add)
        garg = po.work.tile([128, g, NT, 3], F32, tag="garg",
                            name=f"garg{jj}", bufs=7)
        ub = bass.AP(tensor=u.tensor, offset=u.offset,
                     ap=[list(u.ap[0]), [1, g * NT], [0, 3]])
        nc.gpsimd.tensor_tensor(sl(garg), ub, bcol(LNF_C + 3 * jj), OP.mult)
        G = po.work.tile([128, g, NT, 3], F32, tag="G", name=f"G{jj}",
                         bufs=7)
        nc.scalar.activation(sl(G), sl(garg), AF.Exp, scale=1.0)
        nc.gpsimd.tensor_tensor(sl(AE), sl(AE), sl(G), OP.mult)
        yield

    if _DBG_STAGE == 3:
        nc.sync.dma_start(out_d.rearrange("(p t) c -> p t c", t=NT)[:, :, 0],
                          b[:, 0])
        return

    # ---------- refine: the two bracketing real-grid points ----------
    # g0 = max(b, 0), g1 = b+1 <= 999; only b < 0 needs predication.
    d0, tot, rtot = (w3(n) for n in ("d0", "tot", "rtot"))
    pneg = w3("pneg", I32)
    pick0 = w3("pick0", I32)
    AEf, Ssel = (w33(n) for n in ("AEf", "Ssel"))
    pnegk = po.work.tile([128, g, NT, 3], I32, tag="pnegk", name="pnegk")
    pick0k = po.work.tile([128, g, NT, 3], I32, tag="pick0k", name="pick0k")
    SP = po.work.tile([128, 2, g, NT, 3], F32, tag="SP", name="SP")
    SS = po.work.tile([128, 2, g, NT, 3], F32, tag="SS", name="SS")
    it2 = po.work.tile([128, 2, g, NT], F32, tag="it2", name="it2")
    fe3 = po.work.tile([128, g, NT, 2], F32, tag="fe3", name="fe3")

    nc.vector.tensor_scalar(sl(pneg), sl(b), -0.5, None, OP.is_le)

    def m16(t):  # [p, r, t] broadcast over k
        return bass.AP(tensor=t.tensor, offset=t.offset,
                       ap=[list(t.ap[0]), [1, g * NT], [0, 3]])

    nc.vector.tensor_copy(sl(pnegk), m16(pneg))
    nc.vector.tensor_tensor(sl(AEf), sl(AE), bcol(F1_C), OP.mult)
    # g0: AE normally; AE*f1 if b<0.  g1: always AE*f1.
    nc.scalar.activation(SP[:, 0], sl(AE), AF.Copy, scale=1.0)
    nc.vector.copy_predicated(SP[:, 0], sl(pnegk), sl(AEf))
    nc.scalar.activation(SP[:, 1], sl(AEf), AF.Copy, scale=1.0)
    yield
    nc.gpsimd.tensor_tensor(SP, SP, Cfull[:, 0:2],
                            OP.add)
    nc.vector.reciprocal(SS, SP)
    nc.vector.reduce_sum(it2, SS, axis=mybir.AxisListType.X,
                         opt_input=False)
    nc.gpsimd.tensor_tensor(sl(d0), it2[:, 0], it2[:, 1], OP.add)
    # d0 <= d1  <=>  it0 + it1 <= 2*target (i_tot monotone decreasing)
    nc.vector.tensor_scalar(sl(pick0), sl(d0), 2.0 * I_TARGET, None, OP.is_le)
    nc.vector.tensor_copy(sl(pick0k), m16(pick0))
    nc.vector.tensor_copy(sl(Ssel), SS[:, 1])
    nc.vector.copy_predicated(sl(Ssel), sl(pick0k), SS[:, 0])
    nc.vector.reduce_sum(sl(tot), sl(Ssel), axis=mybir.AxisListType.X,
                         opt_input=False)
    nc.vector.reciprocal(sl(rtot), sl(tot))
    nc.gpsimd.tensor_tensor(sl(fe3)[:, :, :, 0], sl(Ssel)[:, :, :, 1], sl(rtot),
                            OP.mult)  # FE_C2H4
    nc.gpsimd.tensor_tensor(sl(fe3)[:, :, :, 1], sl(Ssel)[:, :, :, 0], sl(rtot),
                            OP.mult)  # FE_CO
    for r in range(g):
        nc.sync.dma_start(out_d.rearrange("(p t) c -> p t c", t=NT), fe3[:, r])





def _build(reps=1):
    from contextlib import ExitStack

    nc = bacc.Bacc("TRN2", target_bir_lowering=False, debug=False)
    xT_d = nc.dram_tensor("xT", [5, NPC], F32R, kind="ExternalInput").ap()
    W1_d = nc.dram_tensor("W1", [5, HID], F32R, kind="ExternalInput").ap()
    pack_d = nc.dram_tensor("pack", [HID, 137], F32R, kind="ExternalInput").ap()
    blobz_d = nc.dram_tensor("blobz", [128, NT + NBLOB + 6], F32,
                             kind="ExternalInput").ap()
    out_d = nc.dram_tensor("out", [NPC, 2], F32, kind="ExternalOutput").ap()
    def drive(gens):
        while gens:
            nxt = []
            for gen in gens:
                try:
                    next(gen)
                    nxt.append(gen)
                except StopIteration:
                    pass
            gens = nxt

    with tile.TileContext(nc) as tc:
        with ExitStack() as ctx:
            po = _mk_pools(ctx, tc)
            io = (xT_d, W1_d, pack_d, blobz_d, out_d)
            # group sizes, processed in pairs of groups whose tails interleave
            sizes = []
            left = reps
            while left > 0:
                sizes.append(min(GRPSZ, left))
                left -= sizes[-1]
            done = 0
            for p0 in range(0, len(sizes), 2):
                tails = []
                for g in sizes[p0:p0 + 2]:
                    lat3 = po.work.tile([128, g, NT, 6], F32, tag="lat3",
                                        name="lat3")
                    azlt3 = po.work.tile([128, g, NT], F32, tag="azlt3",
                                         name="azlt3")
                    shared = {}
                    drive([_mlp(tc, po, io, r, g, lat3, azlt3, shared,
                                first=(done == 0 and r == 0))
                           for r in range(g)])
                    tails.append(_tail(tc, po, out_d, g, lat3, azlt3, shared[0]))
                    done += g
                # skew the pair so the two tails' engine phases complement
                if len(tails) == 2:
                    for _ in range(3):
                        try:
                            next(tails[0])
                        except StopIteration:
                            tails = tails[1:]
                            break
                drive(tails)
    nc.compile()
    return nc


_CACHE = {}


def _make_inputs(x, W1, b1, W2, b2, W3, b3, W4, b4, i0, alpha):
    x = np.ascontiguousarray(np.asarray(x, np.float32))
    pack = np.concatenate(
        [np.asarray(W2, np.float32), np.asarray(W3, np.float32),
         np.asarray(W4, np.float32), np.asarray(b1, np.float32)[:, None],
         np.asarray(b2, np.float32)[:, None], np.asarray(b3, np.float32)[:, None]],
        axis=1)
    blob_row = _make_blob_row(i0, alpha)
    b4f = np.asarray(b4, np.float32)
    in_maps = []
    for c in range(NCORES):
        shard = x[c * NPC:(c + 1) * NPC]
        xT = np.ascontiguousarray(
            shard.reshape(128, NT, 5).transpose(2, 1, 0).reshape(5, NPC))
        blobz = np.empty((128, NT + NBLOB + 6), np.float32)
        blobz[:, 0:NT] = shard[:, 3].reshape(128, NT)
        blobz[:, NT:NT + NBLOB] = blob_row
        blobz[:, NT + NBLOB:] = b4f
        in_maps.append({"xT": xT, "W1": np.ascontiguousarray(W1, np.float32),
                        "pack": np.ascontiguousarray(pack), "blobz": blobz})
    return in_maps


def kernel(x, W1, b1, W2, b2, W3, b3, W4, b4, i0, alpha):
    from concourse.bass_utils import run_bass_kernel_spmd

    if "nc" not in _CACHE:
        _CACHE["nc"] = _build()
    nc = _CACHE["nc"]
    in_maps = _make_inputs(x, W1, b1, W2, b2, W3, b3, W4, b4, i0, alpha)
    res = run_bass_kernel_spmd(nc, in_maps, core_ids=list(range(NCORES)))
    return np.concatenate([res.results[c]["out"] for c in range(NCORES)], axis=0)


# revision 78
# speedup vs baseline: 1.1729x; 1.1729x over previous
"""Trainium2 Bass kernel for the CO2-electrolysis surrogate model.

Contract: kernel(**inputs) takes FULL unsharded inputs (x [16384,5], MLP
weights, kinetic params i0/alpha) and returns the FULL [16384,2] output.
Internally: batch is sharded 2048-per-core across 8 NeuronCores (pure data
parallel).

Design (v4):
- x is transposed on the host into xT [5, 2048] (column c = t*128+p holds
  sample s = p*16+t) so the MLP needs no PE transposes. zlt (= x[:,3]), the
  search/kinetics constant table (functions of i0/alpha), and b4 ride in one
  packed [128, 91] input; 4 input DMAs per rep.
- Matmul inputs are float32r end-to-end (1 cycle/row at N=512 vs 4 for fp32).
- Relu reads two-bank PSUM tiles [128, 2x512], split DVE/ACT (tensor_scalar
  is not implemented on the Pool engine).
- Reps are emitted in groups of G: the MLPs run per rep (interleaved
  emission), then ONE grouped tail runs the per-sample physics for all G
  reps with a leading rep axis in every tile - 3x fewer instructions and
  semaphore hops on the serial search chain.
- The voltage search keeps multiplicative state AE[p,r,t,k] = 1/i_kin at the
  current virtual-grid index b; each 4-ary step probes b+{1,2,3}*s via baked
  factor multiplies and updates AE *= exp(sc*s*u) with u = #successful
  probes (no exp of per-sample args; b stays off the critical path).
- The parameter section uses only {exp, ln, abs, relu, copy}, all inside
  activation-table set 6 (natural_log_exp_and_others), loaded once.
  Identities: 1/(1-sigmoid(l)) = 1+e^l, sigmoid(l)^-1.5 = exp(1.5*ln(1+e^-l)).
- b <= 998 always (i_tot(999) < 1e-3 << target for any theta <= 1), so the
  refine step only needs the b < 0 boundary case.
"""

import sys

for _p in ("/opt/trn_rl_repo", "/opt/pypackages"):
    if _p not in sys.path:
        sys.path.insert(0, _p)

import math

import numpy as np

import concourse.bacc as bacc
import concourse.bass as bass
import concourse.tile as tile
from concourse import mybir

F32 = mybir.dt.float32
F32R = mybir.dt.float32r
I32 = mybir.dt.int32
AF = mybir.ActivationFunctionType
OP = mybir.AluOpType

# ---- problem constants (match reference.py) ----
N = 16384
NCORES = 8
NPC = N // NCORES            # 2048 samples per core
NT = NPC // 128              # 16 tiles of 128 samples
HID = 64
GRID = 1000
VMIN, VMAX = -1.25, 0.0
I_TARGET = 200.0
F_CONST = 96485.33
RT = 8.314 * 298.15
D_CO2 = 1.91e-9
C_CO2 = 34.0
E_EQ = (-0.11, 0.08, 0.0)
N_ELEC_CO2 = (2.0, 12.0)
DV = (VMAX - VMIN) / (GRID - 1)
FRT = F_CONST / RT
STEPS = [256, 64, 16, 4, 1]   # 4-ary climb over virtual 1024-grid, b in [-1,1022]
GRPSZ = 3                     # reps per grouped tail
_DBG_STAGE = 0

# blobz layout: [zlt(16) | blob(NBLOB) | b4(6)]
BL0 = NT                      # blob base column inside blobz
LNF_C = 45
CAE_C = 60
F1_C = 63
LN4_C = 66
CIL_C = 67
NBLOB = 70
PK0 = NT + NBLOB + 6         # pack base column inside blobz


def _make_blob_row(i0, alpha):
    """Search/kinetics constants [NBLOB] f32 (functions of i0/alpha)."""
    i0 = np.asarray(i0, np.float64)
    alpha = np.asarray(alpha, np.float64)
    sc = [float(alpha[k] * FRT * DV) for k in range(3)]
    t0 = [float(alpha[k] * FRT * (VMIN - E_EQ[k])) for k in range(3)]
    cols = []
    for s in STEPS:                 # 0:45  probe factors exp(sc*j*s)
        for j in (1, 2, 3):
            for k in range(3):
                cols.append(np.exp(sc[k] * j * s))
    for s in STEPS:                 # 45:60 (only first 3 used: sc_k)
        for k in range(3):
            cols.append(sc[k])
    for k in range(3):              # 60:63 cAE: exp(t0-sc)/i0 (AE at b=-1, pre 1/theta)
        cols.append(np.exp(t0[k] - sc[k]) / float(i0[k]))
    for k in range(3):              # 63:66 f1: one-step factor exp(sc)
        cols.append(np.exp(sc[k]))
    cols.append(math.log(4e-8))     # 66    bias for the r/Kdl exp
    # 67:70  1/i_lim prefactors (3rd species: H2, not transport-limited -> 0)
    for nk in N_ELEC_CO2:
        cols.append(1.0 / (float(np.float32(np.float32(nk) * F_CONST))
                           * C_CO2 * D_CO2))
    cols.append(0.0)
    row = np.asarray(cols, np.float32)
    assert row.size == NBLOB
    return row


class _Pools:
    pass


def _mk_pools(ctx, tc):
    p = _Pools()
    p.io = ctx.enter_context(tc.tile_pool(name="io", bufs=5))
    p.work = ctx.enter_context(tc.tile_pool(name="work", bufs=3))
    p.psum = ctx.enter_context(tc.tile_pool(name="psum", bufs=3, space="PSUM"))
    return p


def _mlp(tc, po, io, r, g, lat3, azlt3, shared, first):
    """Generator: per-rep DMAs + MLP; writes lat into lat3[:, r] and
    |zlt| into azlt3[:, r].  shared[0] collects the rep's blobsb tile."""
    nc = tc.nc
    xT_d, blobz_d, out_d = io

    if first:
        # lock the activation table to set 6 (natural_log_exp_and_others):
        # covers exp/ln/abs/relu/copy -> zero reloads for the whole program
        inst = mybir.InstLoadActFuncSet(
            name=nc.get_next_instruction_name(), act_func_set_id=6, ins=[], outs=[])
        nc.scalar.add_instruction(inst)

    xTsb = po.io.tile([5, 4, 512], F32R, name="xTsb")
    nc.sync.dma_start(xTsb, xT_d.rearrange("k (i n) -> k i n", i=4))
    blobsb_r = po.io.tile([128, PK0 + 137 + 64], F32R, name="blobsb", bufs=6)
    nc.sync.dma_start(blobsb_r, blobz_d)
    blobsb = blobsb_r.bitcast(F32)
    shared[r] = blobsb

    packsb = blobsb_r[0:64, PK0:PK0 + 137]
    W1sb = bass.AP(tensor=blobsb_r.tensor,
                   offset=blobsb_r.offset + PK0 + 137,
                   ap=[[list(blobsb_r.ap[0])[0], 5], [1, 64]])
    W2sb = packsb[:, 0:64]
    W3sb = packsb[:, 64:128]
    W4sb = packsb[:, 128:134]
    biases = [packsb[:, 134 + i:135 + i].bitcast(F32) for i in range(3)]

    def layer(W, movsrc, bias, name, eng):
        ps = [po.psum.tile([128, 2, 512], F32, tag="ps2", name=f"{name}ps{i}")
              for i in range(2)]
        for i in range(4):
            nc.tensor.matmul(ps[i // 2][0:64, i % 2, :], W, movsrc(i))
        h = po.work.tile([64, 4, 512], F32R, tag=name, name=name, bufs=4)
        for i in range(2):
            dst = h[:, 2 * i:2 * i + 2, :]
            src = ps[i][0:64, :, :]
            if eng[i] == "v":
                nc.vector.tensor_scalar(dst, src, bias, 0.0, OP.add, OP.max)
            else:
                nc.scalar.activation(dst, src, AF.Relu, bias=bias, scale=1.0)
        return h

    yield
    h1 = layer(W1sb, lambda i: xTsb[:, i, :], biases[0], "h1", "av")
    nc.scalar.activation(azlt3[:, r], blobsb[:, 0:NT], AF.Abs, scale=1.0)
    yield
    h2 = layer(W2sb, lambda i: h1[:, i, :], biases[1], "h2", "va")
    yield
    h3 = layer(W3sb, lambda i: h2[:, i, :], biases[2], "h3", "aa")
    yield
    h3f = h3.rearrange("p a c -> p (a c)")

    latps = po.psum.tile([128, 96], F32, tag="lat", name="latps", bufs=2)
    for t in range(NT):
        nc.tensor.matmul(latps[:, t * 6:(t + 1) * 6],
                         h3f[:, t * 128:(t + 1) * 128], W4sb)
    b4b = bass.AP(tensor=blobsb.tensor, offset=blobsb.offset + BL0 + NBLOB,
                  ap=[list(blobsb.ap[0]), [0, NT], [1, 6]])
    nc.vector.tensor_tensor(lat3[:, r], latps.rearrange("p (t j) -> p t j", j=6),
                            b4b, OP.add)


def _tail(tc, po, out_d, g, lat3, azlt3, blobsb):
    """Grouped per-sample physics for g reps: parameters, 4-ary climb,
    refine, FE output.  All tiles carry a leading rep axis of size g."""
    nc = tc.nc
    GNT = g * NT

    def bcol(c, n=3):
        """blob columns c..c+n as [p, r(bcast), t(bcast), k] AP"""
        return bass.AP(tensor=blobsb.tensor, offset=blobsb.offset + BL0 + c,
                       ap=[list(blobsb.ap[0]), [0, g * NT], [1, n]])

    if _DBG_STAGE == 1:
        nc.sync.dma_start(out_d.rearrange("(p t) c -> p t c", t=NT),
                          lat3[:, 0, :, 0:2])
        return

    def w3(name, dt=F32):
        return po.work.tile([128, g, NT], dt, tag=name, name=name)

    def w33(name):
        return po.work.tile([128, g, NT, 3], F32, tag=name, name=name)

    a1, e1i, a2, a3, d1, a4, Lt, s5, t6, mm, st, b = (
        w3(n) for n in ("a1", "e1i", "a2", "a3", "d1", "a4", "Lt",
                        "s5", "t6", "mm", "st", "b"))
    C_all, dk, T, iT, t7, AE = (
        w33(n) for n in ("C_all", "dk", "T", "iT", "t7", "AE"))

    def sl(t):
        return t

    l3 = lat3
    nc.scalar.activation(sl(a1), l3[:, :, :, 1], AF.Exp, scale=-1.0)   # e^-l1
    nc.vector.reciprocal(sl(e1i), sl(a1))                              # e^l1
    nc.scalar.activation(sl(a2), sl(a1), AF.Ln, bias=1.0, scale=1.0)   # ln(1+e^-l1)
    nc.scalar.activation(sl(a3), sl(a2), AF.Exp, scale=1.5)            # eps^-1.5
    nc.gpsimd.tensor_tensor(sl(d1), l3[:, :, :, 0], l3[:, :, :, 2], OP.subtract)
    nc.scalar.activation(sl(a4), sl(d1), AF.Exp,
                         bias=blobsb[:, BL0 + LN4_C:BL0 + LN4_C + 1],
                         scale=1.0)                                    # r/Kdl
    nc.vector.scalar_tensor_tensor(sl(Lt), sl(e1i), 1.0, sl(azlt3),
                                   OP.add, OP.mult)
    nc.gpsimd.tensor_tensor(sl(s5), sl(a4), sl(Lt), OP.add)
    nc.gpsimd.tensor_tensor(sl(t6), sl(s5), sl(a3), OP.mult)
    # C_all[:, r, :, k] = t6 * cilim_k  (k=2 blob column is 0 -> H2 term)
    t6b = bass.AP(tensor=t6.tensor, offset=t6.offset,
                  ap=[list(t6.ap[0]), [1, g * NT], [0, 3]])
    nc.vector.tensor_tensor(sl(C_all), t6b, bcol(CIL_C), OP.mult)

    # softmax thetas -> AE init at b=-1
    nc.vector.reduce_max(sl(mm), l3[:, :, :, 3:6], axis=mybir.AxisListType.X,
                         opt_input=False)
    mmb = bass.AP(tensor=mm.tensor, offset=mm.offset,
                  ap=[list(mm.ap[0]), [1, g * NT], [0, 3]])
    nc.vector.tensor_tensor(sl(dk), l3[:, :, :, 3:6], mmb, OP.subtract)
    nc.scalar.activation(sl(T), sl(dk), AF.Exp, scale=2.0)
    nc.vector.reduce_sum(sl(st), sl(T), axis=mybir.AxisListType.X,
                         opt_input=False)
    nc.vector.reciprocal(sl(iT), sl(T))
    yield
    nc.vector.tensor_tensor(sl(t7), sl(iT), bcol(CAE_C), OP.mult)
    stb = bass.AP(tensor=st.tensor, offset=st.offset,
                  ap=[list(st.ap[0]), [1, g * NT], [0, 3]])
    nc.vector.tensor_tensor(sl(AE), sl(t7), stb, OP.mult)     # 1/i_kin at b=-1
    nc.vector.memset(sl(b), -1.0)
    C_b = bass.AP(tensor=C_all.tensor, offset=C_all.offset,
                  ap=[list(C_all.ap[0]), [0, 3], [1, g * NT * 3]])
    C_b2 = bass.AP(tensor=C_all.tensor, offset=C_all.offset,
                   ap=[list(C_all.ap[0]), [0, 2], [1, g * NT * 3]])
    yield

    if _DBG_STAGE == 2:
        o = po.work.tile([128, NT, 2], F32, name="dbg2")
        nc.vector.tensor_copy(o[:, :, 0], C_all[:, 0, :, 0])
        nc.vector.tensor_copy(o[:, :, 1], AE[:, 0, :, 0])
        nc.sync.dma_start(out_d.rearrange("(p t) c -> p t c", t=NT), o)
        return

    # ---------- 4-ary climb ----------
    AEb = bass.AP(tensor=AE.tensor, offset=AE.offset,
                  ap=[list(AE.ap[0]), [0, 3], [1, g * NT * 3]])
    for jj, s in enumerate(STEPS):
        s = float(s)
        AEp = po.work.tile([128, 3, g, NT, 3], F32, tag="AEp",
                           name=f"AEp{jj}", bufs=8)
        fstep = bass.AP(tensor=blobsb.tensor, offset=blobsb.offset + BL0 + 9 * jj,
                        ap=[list(blobsb.ap[0]), [3, 3], [0, g * NT], [1, 3]])
        nc.vector.tensor_tensor(AEp, AEb, fstep, OP.mult)
        P = po.work.tile([128, 3, g, NT, 3], F32, tag="P", name=f"P{jj}",
                         bufs=8)
        nc.gpsimd.tensor_tensor(P, AEp, C_b, OP.add)
        S = po.work.tile([128, 3, g, NT, 3], F32, tag="S", name=f"S{jj}",
                         bufs=8)
        nc.vector.reciprocal(S, P)
        itot = po.work.tile([128, 3, g, NT], F32, tag="it", name=f"it{jj}",
                            bufs=8)
        nc.vector.reduce_sum(itot, S, axis=mybir.AxisListType.X,
                             opt_input=False)
        yield
        # cp[p, j, r, t] = s * (i_tot at probe j >= target):  {0, s}
        cp = po.work.tile([128, 3, g, NT], F32, tag="cp", name=f"cp{jj}",
                          bufs=8)
        nc.vector.tensor_scalar(cp, itot, I_TARGET, s, OP.is_ge, OP.mult)
        # u = s * #successful probes; b += u; AE *= exp(sc*u)
        u = po.work.tile([128, g, NT], F32, tag="u", name=f"u{jj}", bufs=8)
        nc.gpsimd.tensor_tensor(sl(u), cp[:, 0], cp[:, 1], OP.add)
        nc.gpsimd.tensor_tensor(sl(u), sl(u), cp[:, 2], OP.add)
        nc.gpsimd.tensor_tensor(sl(b), sl(b), sl(u), OP.add)
        garg = po.work.tile([128, g, NT, 3], F32, tag="garg",
                            name=f"garg{jj}", bufs=8)
        ub = bass.AP(tensor=u.tensor, offset=u.offset,
                     ap=[list(u.ap[0]), [1, g * NT], [0, 3]])
        nc.gpsimd.tensor_tensor(sl(garg), ub, bcol(LNF_C), OP.mult)
        G = po.work.tile([128, g, NT, 3], F32, tag="G", name=f"G{jj}",
                         bufs=8)
        nc.scalar.activation(sl(G), sl(garg), AF.Exp, scale=1.0)
        nc.gpsimd.tensor_tensor(sl(AE), sl(AE), sl(G), OP.mult)
        yield

    if _DBG_STAGE == 3:
        nc.sync.dma_start(out_d.rearrange("(p t) c -> p t c", t=NT)[:, :, 0],
                          b[:, 0])
        return

    # ---------- refine: the two bracketing real-grid points ----------
    # g0 = max(b, 0), g1 = b+1 <= 999; only b < 0 needs predication.
    d0, tot, rtot = (w3(n) for n in ("d0", "tot", "rtot"))
    pneg = w3("pneg", I32)
    pick0 = w3("pick0", I32)
    AEf, Ssel = (w33(n) for n in ("AEf", "Ssel"))
    pnegk = po.work.tile([128, g, NT, 3], I32, tag="pnegk", name="pnegk")
    pick0k = po.work.tile([128, g, NT, 3], I32, tag="pick0k", name="pick0k")
    SP = po.work.tile([128, 2, g, NT, 3], F32, tag="SP", name="SP")
    SS = po.work.tile([128, 2, g, NT, 3], F32, tag="SS", name="SS")
    it2 = po.work.tile([128, 2, g, NT], F32, tag="it2", name="it2")
    fe3 = po.work.tile([128, g, NT, 2], F32, tag="fe3", name="fe3")

    nc.vector.tensor_scalar(sl(pneg), sl(b), -0.5, None, OP.is_le)

    def m16(t):  # [p, r, t] broadcast over k
        return bass.AP(tensor=t.tensor, offset=t.offset,
                       ap=[list(t.ap[0]), [1, g * NT], [0, 3]])

    nc.vector.tensor_copy(sl(pnegk), m16(pneg))
    nc.vector.tensor_tensor(sl(AEf), sl(AE), bcol(F1_C), OP.mult)
    # g0: AE normally; AE*f1 if b<0.  g1: always AE*f1.
    nc.scalar.activation(SP[:, 0], sl(AE), AF.Copy, scale=1.0)
    nc.vector.copy_predicated(SP[:, 0], sl(pnegk), sl(AEf))
    nc.scalar.activation(SP[:, 1], sl(AEf), AF.Copy, scale=1.0)
    yield
    nc.gpsimd.tensor_tensor(SP, SP, C_b2, OP.add)
    nc.vector.reciprocal(SS, SP)
    nc.vector.reduce_sum(it2, SS, axis=mybir.AxisListType.X,
                         opt_input=False)
    nc.gpsimd.tensor_tensor(sl(d0), it2[:, 0], it2[:, 1], OP.add)
    # d0 <= d1  <=>  it0 + it1 <= 2*target (i_tot monotone decreasing)
    nc.vector.tensor_scalar(sl(pick0), sl(d0), 2.0 * I_TARGET, None, OP.is_le)
    nc.vector.tensor_copy(sl(pick0k), m16(pick0))
    nc.vector.tensor_copy(sl(Ssel), SS[:, 1])
    nc.vector.copy_predicated(sl(Ssel), sl(pick0k), SS[:, 0])
    nc.vector.reduce_sum(sl(tot), sl(Ssel), axis=mybir.AxisListType.X,
                         opt_input=False)
    nc.vector.reciprocal(sl(rtot), sl(tot))
    nc.gpsimd.tensor_tensor(sl(fe3)[:, :, :, 0], sl(Ssel)[:, :, :, 1], sl(rtot),
                            OP.mult)  # FE_C2H4
    nc.gpsimd.tensor_tensor(sl(fe3)[:, :, :, 1], sl(Ssel)[:, :, :, 0], sl(rtot),
                            OP.mult)  # FE_CO
    for r in range(g):
        nc.sync.dma_start(out_d.rearrange("(p t) c -> p t c", t=NT), fe3[:, r])





def _build(reps=1):
    from contextlib import ExitStack

    nc = bacc.Bacc("TRN2", target_bir_lowering=False, debug=False)
    xT_d = nc.dram_tensor("xT", [5, NPC], F32R, kind="ExternalInput").ap()
    blobz_d = nc.dram_tensor("blobz", [128, PK0 + 137 + 64], F32R,
                             kind="ExternalInput").ap()
    out_d = nc.dram_tensor("out", [NPC, 2], F32, kind="ExternalOutput").ap()
    def drive(gens):
        while gens:
            nxt = []
            for gen in gens:
                try:
                    next(gen)
                    nxt.append(gen)
                except StopIteration:
                    pass
            gens = nxt

    with tile.TileContext(nc) as tc:
        with ExitStack() as ctx:
            po = _mk_pools(ctx, tc)
            io = (xT_d, blobz_d, out_d)
            # group sizes, processed in pairs of groups whose tails interleave
            sizes = []
            left = reps
            while left > 0:
                sizes.append(min(GRPSZ, left))
                left -= sizes[-1]
            done = 0
            for p0 in range(0, len(sizes), 2):
                tails = []
                for g in sizes[p0:p0 + 2]:
                    lat3 = po.work.tile([128, g, NT, 6], F32, tag="lat3",
                                        name="lat3")
                    azlt3 = po.work.tile([128, g, NT], F32, tag="azlt3",
                                         name="azlt3")
                    shared = {}
                    drive([_mlp(tc, po, io, r, g, lat3, azlt3, shared,
                                first=(done == 0 and r == 0))
                           for r in range(g)])
                    tails.append(_tail(tc, po, out_d, g, lat3, azlt3, shared[0]))
                    done += g
                drive(tails)
    nc.compile()
    return nc


_CACHE = {}


def _make_inputs(x, W1, b1, W2, b2, W3, b3, W4, b4, i0, alpha):
    x = np.ascontiguousarray(np.asarray(x, np.float32))
    pack = np.concatenate(
        [np.asarray(W2, np.float32), np.asarray(W3, np.float32),
         np.asarray(W4, np.float32), np.asarray(b1, np.float32)[:, None],
         np.asarray(b2, np.float32)[:, None], np.asarray(b3, np.float32)[:, None]],
        axis=1)
    blob_row = _make_blob_row(i0, alpha)
    b4f = np.asarray(b4, np.float32)
    in_maps = []
    for c in range(NCORES):
        shard = x[c * NPC:(c + 1) * NPC]
        xT = np.ascontiguousarray(
            shard.reshape(128, NT, 5).transpose(2, 1, 0).reshape(5, NPC))
        blobz = np.zeros((128, PK0 + 137 + 64), np.float32)
        blobz[:, 0:NT] = shard[:, 3].reshape(128, NT)
        blobz[:, NT:NT + NBLOB] = blob_row
        blobz[:, NT + NBLOB:PK0] = b4f
        blobz[0:64, PK0:PK0 + 137] = pack
        blobz[0:5, PK0 + 137:] = np.asarray(W1, np.float32)
        in_maps.append({"xT": xT, "blobz": blobz})
    return in_maps


def kernel(x, W1, b1, W2, b2, W3, b3, W4, b4, i0, alpha):
    from concourse.bass_utils import run_bass_kernel_spmd

    if "nc" not in _CACHE:
        _CACHE["nc"] = _build()
    nc = _CACHE["nc"]
    in_maps = _make_inputs(x, W1, b1, W2, b2, W3, b3, W4, b4, i0, alpha)
    res = run_bass_kernel_spmd(nc, in_maps, core_ids=list(range(NCORES)))
    return np.concatenate([res.results[c]["out"] for c in range(NCORES)], axis=0)
